# revision 3
# baseline (speedup 1.0000x reference)
"""Multi-head attention (B=2, S=2048, D=2048, H=16) on 8 Trainium2 NeuronCores.

Sharding: 2D grid (4 head-groups x 2 batches). Core c = (hg=c//2, b=c%2)
computes heads [4*hg, 4*hg+4) of batch b end-to-end:
  - q/k/v projections for its 4 heads over its batch's 2048 tokens
    (host pre-transposes activations to [D, S] and weight slices to
    [D, 512] so every matmul contraction sits on the partition dim),
  - full attention for its 4 (b, h) pairs in transposed-score space
    (scoresT[ki,qi] -> exp -> v.T @ expT accumulation), softmax
    denominator via DVE tile-accumulate + PE transpose + row reduce,
  - row-parallel output projection producing a [2048, 2048] partial
    that the host sums over the 4 head-groups (plus bo).

All matmuls run as float32r (replicated-fp32, 1 cycle/row on the PE at
moving-dim >= 256; ~1.5e-4 max rel err vs fp64, same as TRN2's fp32 path).
"""
import math
import numpy as np
from contextlib import ExitStack

import concourse.bacc as bacc
import concourse.mybir as mybir
import concourse.tile as tile
from concourse.bass_utils import run_bass_kernel_spmd
from concourse.masks import make_identity

F32 = mybir.dt.float32
F32R = mybir.dt.float32r
AF = mybir.ActivationFunctionType
AX = mybir.AxisListType

B, S, D, H = 2, 2048, 2048, 16
HD = D // H            # 128
NCORES = 8
HGROUPS = 4            # head groups
NH = H // HGROUPS      # 4 heads per core
FEAT = NH * HD         # 512 projected features per core
TOK = S                # tokens per core (one batch)
DT = D // 128          # 16 contraction tiles
IT = TOK // 128        # 16 token tiles
IB = 512               # i-block width (projections / out-proj moving dim)
NB = TOK // IB         # 4 i-blocks
QB = 256               # qi-block width (attention moving dim)
NQ = TOK // QB         # 8 qi-blocks
KT = TOK // 128        # 16 ki tiles

_NC_CACHE = {}


def _build_nc():
    nc = bacc.Bacc("TRN2", target_bir_lowering=False, debug=False)

    xq = nc.dram_tensor("xq", [D, TOK], F32R, kind="ExternalInput").ap()
    xk = nc.dram_tensor("xk", [D, TOK], F32R, kind="ExternalInput").ap()
    xv = nc.dram_tensor("xv", [D, TOK], F32R, kind="ExternalInput").ap()
    wq = nc.dram_tensor("wq", [D, FEAT], F32R, kind="ExternalInput").ap()
    wk = nc.dram_tensor("wk", [D, FEAT], F32R, kind="ExternalInput").ap()
    wv = nc.dram_tensor("wv", [D, FEAT], F32R, kind="ExternalInput").ap()
    wo = nc.dram_tensor("wo", [FEAT, D], F32R, kind="ExternalInput").ap()
    bqs = nc.dram_tensor("bqs", [128, NH], F32, kind="ExternalInput").ap()
    bks = nc.dram_tensor("bks", [128, NH], F32, kind="ExternalInput").ap()
    bvb = nc.dram_tensor("bvb", [128, FEAT], F32, kind="ExternalInput").ap()
    out_part = nc.dram_tensor("out_part", [TOK, D], F32, kind="ExternalOutput").ap()

    with tile.TileContext(nc) as tc, ExitStack() as ctx:
        wide = ctx.enter_context(tc.tile_pool(name="wide", bufs=9))
        stream = ctx.enter_context(tc.tile_pool(name="stream", bufs=18))
        vpool = ctx.enter_context(tc.tile_pool(name="vpool", bufs=16))
        wpool = ctx.enter_context(tc.tile_pool(name="wpool", bufs=16))
        mpool = ctx.enter_context(tc.tile_pool(name="mpool", bufs=1))
        spool = ctx.enter_context(tc.tile_pool(name="spool", bufs=3))
        opool = ctx.enter_context(tc.tile_pool(name="opool", bufs=3))
        psum = ctx.enter_context(tc.tile_pool(name="psum", bufs=8, space="PSUM"))

        ident = mpool.tile([128, 128], F32, tag="ident")
        make_identity(nc, ident[:])
        bq_sb = mpool.tile([128, NH], F32, tag="bq")
        nc.sync.dma_start(bq_sb[:], bqs[:])
        bk_sb = mpool.tile([128, NH], F32, tag="bk")
        nc.sync.dma_start(bk_sb[:], bks[:])
        bv_sb = mpool.tile([128, FEAT], F32, tag="bv")
        nc.sync.dma_start(bv_sb[:], bvb[:])

        # ---- phase 1: projections -------------------------------------
        # qT/kT: [feat, tok] (4 head tiles of [128, 2048]); v: [tok, feat].
        qT = []
        kT = []
        v_tiles = []

        for which, xdram, wdram in (("q", xq, wq), ("k", xk, wk), ("v", xv, wv)):
            w_sb = []
            for dt_i in range(DT):
                w_t = wpool.tile([128, FEAT], F32R, tag="w", name=f"w{which}{dt_i}")
                nc.sync.dma_start(w_t[:], wdram[dt_i * 128:(dt_i + 1) * 128, :])
                w_sb.append(w_t)
            if which == "q":
                dst = qT
            elif which == "k":
                dst = kT
            if which in ("q", "k"):
                for j in range(NH):
                    t = wide.tile([128, TOK], F32R, tag="wide", name=f"{which}T{j}")
                    dst.append(t)

            for blk in range(NB):
                xt = []
                for dt_i in range(DT):
                    x_t = stream.tile([128, IB], F32R, tag="s5",
                                      name=f"x{which}{blk}_{dt_i}")
                    nc.sync.dma_start(
                        x_t[:], xdram[dt_i * 128:(dt_i + 1) * 128,
                                      blk * IB:(blk + 1) * IB])
                    xt.append(x_t)
                if which in ("q", "k"):
                    # out tile [feat128, IB] = sum_d W[d,jslice].T @ xT[d,blk]
                    for j in range(NH):
                        ps = psum.tile([128, IB], F32, tag="ps",
                                       name=f"ps{which}{blk}{j}")
                        for dt_i in range(DT):
                            nc.tensor.matmul(
                                ps[:],
                                w_sb[dt_i][:, j * 128:(j + 1) * 128],
                                xt[dt_i][:],
                                start=(dt_i == 0), stop=(dt_i == DT - 1))
                        bias = bq_sb if which == "q" else bk_sb
                        scale = (1.0 / math.sqrt(HD)) if which == "q" else 1.0
                        nc.scalar.activation(
                            dst[j][:, blk * IB:(blk + 1) * IB], ps[:],
                            AF.Identity, bias=bias[:, j:j + 1], scale=scale)
                else:
                    # v: out tile [tok128, FEAT] = sum_d xT[d,itile].T @ W[d,:]
                    for it_l in range(IB // 128):
                        ps = psum.tile([128, FEAT], F32, tag="ps",
                                       name=f"psv{blk}{it_l}")
                        for dt_i in range(DT):
                            nc.tensor.matmul(
                                ps[:],
                                xt[dt_i][:, it_l * 128:(it_l + 1) * 128],
                                w_sb[dt_i][:],
                                start=(dt_i == 0), stop=(dt_i == DT - 1))
                        v_t = vpool.tile([128, FEAT], F32R, tag="v",
                                         name=f"v{blk}{it_l}")
                        nc.vector.tensor_add(v_t[:], ps[:], bv_sb[:])
                        v_tiles.append(v_t)

        # ---- phase 2: attention per head ------------------------------
        ctxT = []
        for h in range(NH):
            qh = qT[h]
            kh = kT[h]
            ch = wide.tile([128, TOK], F32R, tag="wide", name=f"ctxT{h}")
            ctxT.append(ch)
            for qb in range(NQ):
                qs = qh[:, qb * QB:(qb + 1) * QB]
                avp = psum.tile([128, QB], F32, tag="ps", name=f"avp{h}{qb}")
                acc = spool.tile([128, QB], F32, tag="acc", name=f"acc{h}{qb}")
                for t in range(KT):
                    sps = psum.tile([128, QB], F32, tag="ps", name=f"sp{h}{qb}{t}")
                    nc.tensor.matmul(sps[:], kh[:, t * 128:(t + 1) * 128], qs,
                                     start=True, stop=True)
                    et = stream.tile([128, QB], F32R, tag="s5",
                                     name=f"e{h}{qb}{t}")
                    nc.scalar.activation(et[:], sps[:], AF.Exp)
                    nc.tensor.matmul(avp[:],
                                     v_tiles[t][:, h * 128:(h + 1) * 128], et[:],
                                     start=(t == 0), stop=(t == KT - 1))
                    if t == 0:
                        nc.vector.tensor_copy(acc[:], et[:].bitcast(F32))
                    else:
                        nc.vector.tensor_add(acc[:], acc[:], et[:].bitcast(F32))
                # denominator: transpose acc chunks, reduce over free dim
                tp1 = psum.tile([128, QB], F32, tag="ps", name=f"tp1{h}{qb}")
                for c in range(QB // 128):
                    nc.tensor.transpose(tp1[:, c * 128:(c + 1) * 128],
                                        acc[:, c * 128:(c + 1) * 128], ident[:])
                den = spool.tile([128, QB // 128], F32, tag="den",
                                 name=f"den{h}{qb}")
                for c in range(QB // 128):
                    nc.vector.reduce_sum(den[:, c:c + 1],
                                         tp1[:, c * 128:(c + 1) * 128], axis=AX.X)
                recip = spool.tile([128, QB // 128], F32, tag="recip",
                                   name=f"rc{h}{qb}")
                nc.vector.reciprocal(recip[:], den[:])
                # evacuate attention out, normalize in transposed land,
                # transpose back to ctxT layout
                avsb = stream.tile([128, QB], F32, tag="s5", name=f"av{h}{qb}")
                nc.scalar.activation(avsb[:], avp[:], AF.Copy)
                tp2 = psum.tile([128, QB], F32, tag="ps", name=f"tp2{h}{qb}")
                for c in range(QB // 128):
                    nc.tensor.transpose(tp2[:, c * 128:(c + 1) * 128],
                                        avsb[:, c * 128:(c + 1) * 128], ident[:])
                csb = spool.tile([128, QB], F32, tag="ctxsb", name=f"cs{h}{qb}")
                for c in range(QB // 128):
                    nc.vector.tensor_scalar_mul(csb[:, c * 128:(c + 1) * 128],
                                                tp2[:, c * 128:(c + 1) * 128],
                                                recip[:, c:c + 1])
                tp3 = psum.tile([128, QB], F32, tag="ps", name=f"tp3{h}{qb}")
                for c in range(QB // 128):
                    nc.tensor.transpose(tp3[:, c * 128:(c + 1) * 128],
                                        csb[:, c * 128:(c + 1) * 128], ident[:])
                nc.scalar.activation(ch[:, qb * QB:(qb + 1) * QB], tp3[:],
                                     AF.Copy)

        # ---- phase 3: output projection (row-parallel partial) ---------
        wo_sb = []
        for h in range(NH):
            row = []
            for jb in range(D // IB):
                w_t = wpool.tile([128, IB], F32R, tag="w", name=f"wo{h}{jb}")
                nc.sync.dma_start(
                    w_t[:], wo[h * 128:(h + 1) * 128, jb * IB:(jb + 1) * IB])
                row.append(w_t)
            wo_sb.append(row)

        for it_i in range(IT):
            for jb in range(D // IB):
                ps = psum.tile([128, IB], F32, tag="ps", name=f"po{it_i}{jb}")
                for h in range(NH):
                    nc.tensor.matmul(ps[:],
                                     ctxT[h][:, it_i * 128:(it_i + 1) * 128],
                                     wo_sb[h][jb][:],
                                     start=(h == 0), stop=(h == NH - 1))
                osb = opool.tile([128, IB], F32, tag="o", name=f"o{it_i}{jb}")
                if jb % 2 == 0:
                    nc.vector.tensor_copy(osb[:], ps[:])
                else:
                    nc.scalar.activation(osb[:], ps[:], AF.Copy)
                nc.sync.dma_start(
                    out_part[it_i * 128:(it_i + 1) * 128,
                             jb * IB:(jb + 1) * IB], osb[:])

    nc.compile()
    return nc


def _get_nc():
    if "nc" not in _NC_CACHE:
        _NC_CACHE["nc"] = _build_nc()
    return _NC_CACHE["nc"]


def _prep_inputs(query, key_, value, Wq, bq, Wk, bk, Wv, bv, Wo, bo):
    f32 = np.float32
    query = np.asarray(query, f32)
    key_ = np.asarray(key_, f32)
    value = np.asarray(value, f32)
    Wq, bq = np.asarray(Wq, f32), np.asarray(bq, f32)
    Wk, bk = np.asarray(Wk, f32), np.asarray(bk, f32)
    Wv, bv = np.asarray(Wv, f32), np.asarray(bv, f32)
    Wo = np.asarray(Wo, f32)

    sc = f32(1.0 / math.sqrt(HD))
    xqT = [np.ascontiguousarray(query[b].T) for b in range(B)]
    xkT = [np.ascontiguousarray(key_[b].T) for b in range(B)]
    xvT = [np.ascontiguousarray(value[b].T) for b in range(B)]

    per_hg = []
    for hg in range(HGROUPS):
        rows = slice(hg * FEAT, (hg + 1) * FEAT)
        per_hg.append(dict(
            wq=np.ascontiguousarray(Wq[rows, :].T),
            wk=np.ascontiguousarray(Wk[rows, :].T),
            wv=np.ascontiguousarray(Wv[rows, :].T),
            wo=np.ascontiguousarray(Wo[:, rows].T),
            bqs=np.ascontiguousarray((bq[rows] * sc).reshape(NH, 128).T),
            bks=np.ascontiguousarray(bk[rows].reshape(NH, 128).T),
            bvb=np.ascontiguousarray(np.broadcast_to(bv[rows], (128, FEAT))),
        ))

    in_maps = []
    for c in range(NCORES):
        hg, b = divmod(c, B)
        m = dict(per_hg[hg])
        m["xq"] = xqT[b]
        m["xk"] = xkT[b]
        m["xv"] = xvT[b]
        in_maps.append(m)
    return in_maps


def run_on_device(query, key_, value, Wq, bq, Wk, bk, Wv, bv, Wo, bo,
                  **spmd_kwargs):
    """Build+run; returns (BassKernelResults, assembled full output)."""
    nc = _get_nc()
    in_maps = _prep_inputs(query, key_, value, Wq, bq, Wk, bk, Wv, bv, Wo, bo)
    res = run_bass_kernel_spmd(nc, in_maps, core_ids=list(range(NCORES)),
                               **spmd_kwargs)
    bo = np.asarray(bo, np.float32)
    out = np.zeros((B, S, D), np.float32)
    for c in range(NCORES):
        hg, b = divmod(c, B)
        out[b] += res.results[c]["out_part"]
    out += bo
    return res, out


def kernel(query, key_, value, Wq, bq, Wk, bk, Wv, bv, Wo, bo):
    _, out = run_on_device(query, key_, value, Wq, bq, Wk, bk, Wv, bv, Wo, bo)
    return out


# revision 9
# speedup vs baseline: 1.2890x; 1.2890x over previous
"""Multi-head attention (B=2, S=2048, D=2048, H=16) on 8 Trainium2 NeuronCores.

Sharding: 2D grid (4 head-groups x 2 batches). Core c = (hg=c//2, b=c%2)
computes heads [4*hg, 4*hg+4) of batch b end-to-end:
  - q/k/v projections for its 4 heads over its batch's 2048 tokens
    (host pre-transposes activations to [D, S] and weight slices to
    [D, 512] so every matmul contraction sits on the partition dim),
  - full attention for its 4 (b, h) pairs in transposed-score space
    (scoresT[ki,qi] -> exp -> v.T @ expT accumulation); the softmax
    denominator rides the PE as a ones-stationary matmul accumulating
    [1, qi] sums in PSUM, reshaped to per-partition scalars via an
    SBUF->SBUF DMA; blocks are software-pipelined so the PE consumes
    block B-1's exp tiles while ACT produces block B's,
  - row-parallel output projection producing a [2048, 2048] partial
    that the host sums over the 4 head-groups (plus bo).

All matmuls run as float32r (replicated-fp32; ~1.5e-4 max rel err vs
fp64, same as TRN2's fp32 path, at a fraction of the cost).
"""
import math
import numpy as np
from contextlib import ExitStack

import concourse.bacc as bacc
import concourse.mybir as mybir
import concourse.tile as tile
from concourse.bass_utils import run_bass_kernel_spmd
from concourse.masks import make_identity

F32 = mybir.dt.float32
F32R = mybir.dt.float32r
AF = mybir.ActivationFunctionType
AX = mybir.AxisListType

B, S, D, H = 2, 2048, 2048, 16
HD = D // H            # 128
NCORES = 8
HGROUPS = 4            # head groups
NH = H // HGROUPS      # 4 heads per core
FEAT = NH * HD         # 512 projected features per core
TOK = S                # tokens per core (one batch)
DT = D // 128          # 16 contraction tiles
IT = TOK // 128        # 16 token tiles
IB = 512               # i-block width (projections / out-proj moving dim)
NB = TOK // IB         # 4 i-blocks
QB = 512               # qi-block width (attention moving dim)
NQ = TOK // QB         # 4 qi-blocks
KT = TOK // 128        # 16 ki tiles
QC = QB // 128         # 4 qi chunks per block

_NC_CACHE = {}


def _build_nc():
    nc = bacc.Bacc("TRN2", target_bir_lowering=False, debug=False)

    xq = nc.dram_tensor("xq", [D, TOK], F32R, kind="ExternalInput").ap()
    xk = nc.dram_tensor("xk", [D, TOK], F32R, kind="ExternalInput").ap()
    xv = nc.dram_tensor("xv", [D, TOK], F32R, kind="ExternalInput").ap()
    wq = nc.dram_tensor("wq", [D, FEAT], F32R, kind="ExternalInput").ap()
    wk = nc.dram_tensor("wk", [D, FEAT], F32R, kind="ExternalInput").ap()
    wv = nc.dram_tensor("wv", [D, FEAT], F32R, kind="ExternalInput").ap()
    wo = nc.dram_tensor("wo", [FEAT, D], F32R, kind="ExternalInput").ap()
    bqs = nc.dram_tensor("bqs", [128, NH], F32, kind="ExternalInput").ap()
    bks = nc.dram_tensor("bks", [128, NH], F32, kind="ExternalInput").ap()
    bvb = nc.dram_tensor("bvb", [128, FEAT], F32, kind="ExternalInput").ap()
    out_part = nc.dram_tensor("out_part", [TOK, D], F32, kind="ExternalOutput").ap()

    with tile.TileContext(nc) as tc, ExitStack() as ctx:
        wide = ctx.enter_context(tc.tile_pool(name="wide", bufs=9))
        stream = ctx.enter_context(tc.tile_pool(name="stream", bufs=18))
        vpool = ctx.enter_context(tc.tile_pool(name="vpool", bufs=16))
        wpool = ctx.enter_context(tc.tile_pool(name="wpool", bufs=16))
        mpool = ctx.enter_context(tc.tile_pool(name="mpool", bufs=1))
        spool = ctx.enter_context(tc.tile_pool(name="spool", bufs=2))
        opool = ctx.enter_context(tc.tile_pool(name="opool", bufs=2))
        psum = ctx.enter_context(tc.tile_pool(name="psum", bufs=4, space="PSUM"))

        ident = mpool.tile([128, 128], F32, tag="ident")
        make_identity(nc, ident[:])
        ones_f = mpool.tile([128, 1], F32, tag="onesf")
        nc.vector.memset(ones_f[:], 1.0)
        ones = mpool.tile([128, 1], F32R, tag="ones")
        nc.vector.tensor_copy(ones[:], ones_f[:])
        bq_sb = mpool.tile([128, NH], F32, tag="bq")
        nc.sync.dma_start(bq_sb[:], bqs[:])
        bk_sb = mpool.tile([128, NH], F32, tag="bk")
        nc.sync.dma_start(bk_sb[:], bks[:])
        bv_sb = mpool.tile([128, FEAT], F32, tag="bv")
        nc.sync.dma_start(bv_sb[:], bvb[:])

        # PE warmup: junk matmuls with no data deps keep the HAM activity
        # monitor at full clock while the first DMAs land.
        wu = psum.tile([128, 128], F32, tag="sc", name="wu")
        for _ in range(24):
            nc.tensor.matmul(wu[:], ident[:], ident[:], start=True, stop=True)

        # ---- phase 1: projections -------------------------------------
        qT = []
        kT = []
        v_tiles = []

        for which, xdram, wdram in (("q", xq, wq), ("k", xk, wk), ("v", xv, wv)):
            w_sb = []
            for dt_i in range(DT):
                w_t = wpool.tile([128, FEAT], F32R, tag="w", name=f"w{which}{dt_i}")
                nc.sync.dma_start(w_t[:], wdram[dt_i * 128:(dt_i + 1) * 128, :])
                w_sb.append(w_t)
            if which in ("q", "k"):
                dst = qT if which == "q" else kT
                for j in range(NH):
                    t = wide.tile([128, TOK], F32R, tag="wide", name=f"{which}T{j}")
                    dst.append(t)

            for blk in range(NB):
                xt = []
                for dt_i in range(DT):
                    x_t = stream.tile([128, IB], F32R, tag="s5",
                                      name=f"x{which}{blk}_{dt_i}")
                    nc.sync.dma_start(
                        x_t[:], xdram[dt_i * 128:(dt_i + 1) * 128,
                                      blk * IB:(blk + 1) * IB])
                    xt.append(x_t)
                if which in ("q", "k"):
                    # out tile [feat128, IB] = sum_d W[d,jslice].T @ xT[d,blk]
                    bias = bq_sb if which == "q" else bk_sb
                    scale = (1.0 / math.sqrt(HD)) if which == "q" else 1.0
                    for j in range(NH):
                        ps = psum.tile([128, IB], F32, tag="sc",
                                       name=f"ps{which}{blk}{j}")
                        for dt_i in range(DT):
                            nc.tensor.matmul(
                                ps[:],
                                w_sb[dt_i][:, j * 128:(j + 1) * 128],
                                xt[dt_i][:],
                                start=(dt_i == 0), stop=(dt_i == DT - 1))
                        nc.vector.tensor_scalar(
                            dst[j][:, blk * IB:(blk + 1) * IB], ps[:],
                            float(scale), bias[:, j:j + 1],
                            op0=mybir.AluOpType.mult, op1=mybir.AluOpType.add)
                else:
                    # v: out tile [tok128, FEAT] = sum_d xT[d,itile].T @ W[d,:]
                    for it_l in range(IB // 128):
                        ps = psum.tile([128, FEAT], F32, tag="sc",
                                       name=f"psv{blk}{it_l}")
                        for dt_i in range(DT):
                            nc.tensor.matmul(
                                ps[:],
                                xt[dt_i][:, it_l * 128:(it_l + 1) * 128],
                                w_sb[dt_i][:],
                                start=(dt_i == 0), stop=(dt_i == DT - 1))
                        v_t = vpool.tile([128, FEAT], F32R, tag="v",
                                         name=f"v{blk}{it_l}")
                        nc.vector.tensor_add(v_t[:], ps[:], bv_sb[:])
                        v_tiles.append(v_t)

        # ---- phase 2: attention, software-pipelined over qi-blocks ----
        ctxT = [wide.tile([128, TOK], F32R, tag="wide", name=f"ctxT{h}")
                for h in range(NH)]

        def block_pass(cur, prev):
            """Interleave at tile granularity: score+exp for block `cur`
            with attn@v + denominator matmuls consuming block `prev`'s exp
            tiles, so the PE never waits on the current block's ACT work
            and prev's exp tiles free up as the loop advances."""
            ets = []
            if cur is not None:
                h, qb = cur
                qs = qT[h][:, qb * QB:(qb + 1) * QB]
                kh = kT[h]
            if prev is not None:
                ph, pqb, pets = prev
                avp = psum.tile([128, QB], F32, tag="post", name=f"avp{ph}{pqb}")
                denp = psum.tile([1, QB], F32, tag="post", name=f"dnp{ph}{pqb}")
            for t in range(KT):
                if cur is not None:
                    sps = psum.tile([128, QB], F32, tag="sc",
                                    name=f"sp{h}{qb}{t}")
                    nc.tensor.matmul(sps[:], kh[:, t * 128:(t + 1) * 128], qs,
                                     start=True, stop=True)
                    et = stream.tile([128, QB], F32R, tag="s5",
                                     name=f"e{h}{qb}{t}")
                    nc.scalar.activation(et[:], sps[:], AF.Exp)
                    ets.append(et)
                if prev is not None:
                    nc.tensor.matmul(
                        avp[:], v_tiles[t][:, ph * 128:(ph + 1) * 128],
                        pets[t][:], start=(t == 0), stop=(t == KT - 1))
                    nc.tensor.matmul(denp[:], ones[:], pets[t][:],
                                     start=(t == 0), stop=(t == KT - 1))
            if prev is None:
                return ets
            # posts for prev: denominator reshape + normalize + ctxT
            den_sb = spool.tile([1, QB], F32, tag="denr", name=f"dr{ph}{pqb}")
            nc.scalar.activation(den_sb[:], denp[:], AF.Copy)
            dct = psum.tile([128, QC], F32, tag="post", name=f"dct{ph}{pqb}")
            for c in range(QC):
                nc.tensor.transpose(dct[:, c:c + 1],
                                    den_sb[:1, c * 128:(c + 1) * 128],
                                    ident[:1, :1])
            recip = spool.tile([128, QC], F32, tag="recip", name=f"rc{ph}{pqb}")
            nc.vector.reciprocal(recip[:], dct[:])

            avsb = spool.tile([128, QB], F32, tag="avsb", name=f"av{ph}{pqb}")
            nc.vector.tensor_copy(avsb[:], avp[:])
            tp2 = psum.tile([128, QB], F32, tag="post", name=f"tp2{ph}{pqb}")
            for c in range(QC):
                nc.tensor.transpose(tp2[:, c * 128:(c + 1) * 128],
                                    avsb[:, c * 128:(c + 1) * 128], ident[:])
            csb = spool.tile([128, QB], F32, tag="ctxsb", name=f"cs{ph}{pqb}")
            for c in range(QC):
                nc.vector.tensor_scalar_mul(csb[:, c * 128:(c + 1) * 128],
                                            tp2[:, c * 128:(c + 1) * 128],
                                            recip[:, c:c + 1])
            tp3 = psum.tile([128, QB], F32, tag="post", name=f"tp3{ph}{pqb}")
            for c in range(QC):
                nc.tensor.transpose(tp3[:, c * 128:(c + 1) * 128],
                                    csb[:, c * 128:(c + 1) * 128], ident[:])
            nc.scalar.activation(ctxT[ph][:, pqb * QB:(pqb + 1) * QB], tp3[:],
                                 AF.Copy)
            return ets

        prev = None
        for h in range(NH):
            for qb in range(NQ):
                ets = block_pass((h, qb), prev)
                prev = (h, qb, ets)
        block_pass(None, prev)

        # ---- phase 3: output projection (row-parallel partial) ---------
        wo_sb = []
        for h in range(NH):
            row = []
            for jb in range(D // IB):
                w_t = wpool.tile([128, IB], F32R, tag="w", name=f"wo{h}{jb}")
                nc.sync.dma_start(
                    w_t[:], wo[h * 128:(h + 1) * 128, jb * IB:(jb + 1) * IB])
                row.append(w_t)
            wo_sb.append(row)

        for it_i in range(IT):
            for jb in range(D // IB):
                ps = psum.tile([128, IB], F32, tag="sc", name=f"po{it_i}{jb}")
                for h in range(NH):
                    nc.tensor.matmul(ps[:],
                                     ctxT[h][:, it_i * 128:(it_i + 1) * 128],
                                     wo_sb[h][jb][:],
                                     start=(h == 0), stop=(h == NH - 1))
                osb = opool.tile([128, IB], F32, tag="o", name=f"o{it_i}{jb}")
                if jb % 2 == 0:
                    nc.vector.tensor_copy(osb[:], ps[:])
                else:
                    nc.scalar.activation(osb[:], ps[:], AF.Copy)
                nc.sync.dma_start(
                    out_part[it_i * 128:(it_i + 1) * 128,
                             jb * IB:(jb + 1) * IB], osb[:])

    nc.compile()
    return nc


def _get_nc():
    if "nc" not in _NC_CACHE:
        _NC_CACHE["nc"] = _build_nc()
    return _NC_CACHE["nc"]


def _prep_inputs(query, key_, value, Wq, bq, Wk, bk, Wv, bv, Wo, bo):
    f32 = np.float32
    query = np.asarray(query, f32)
    key_ = np.asarray(key_, f32)
    value = np.asarray(value, f32)
    Wq, bq = np.asarray(Wq, f32), np.asarray(bq, f32)
    Wk, bk = np.asarray(Wk, f32), np.asarray(bk, f32)
    Wv, bv = np.asarray(Wv, f32), np.asarray(bv, f32)
    Wo = np.asarray(Wo, f32)

    sc = f32(1.0 / math.sqrt(HD))
    xqT = [np.ascontiguousarray(query[b].T) for b in range(B)]
    xkT = [np.ascontiguousarray(key_[b].T) for b in range(B)]
    xvT = [np.ascontiguousarray(value[b].T) for b in range(B)]

    per_hg = []
    for hg in range(HGROUPS):
        rows = slice(hg * FEAT, (hg + 1) * FEAT)
        per_hg.append(dict(
            wq=np.ascontiguousarray(Wq[rows, :].T),
            wk=np.ascontiguousarray(Wk[rows, :].T),
            wv=np.ascontiguousarray(Wv[rows, :].T),
            wo=np.ascontiguousarray(Wo[:, rows].T),
            bqs=np.ascontiguousarray((bq[rows] * sc).reshape(NH, 128).T),
            bks=np.ascontiguousarray(bk[rows].reshape(NH, 128).T),
            bvb=np.ascontiguousarray(np.broadcast_to(bv[rows], (128, FEAT))),
        ))

    in_maps = []
    for c in range(NCORES):
        hg, b = divmod(c, B)
        m = dict(per_hg[hg])
        m["xq"] = xqT[b]
        m["xk"] = xkT[b]
        m["xv"] = xvT[b]
        in_maps.append(m)
    return in_maps


def run_on_device(query, key_, value, Wq, bq, Wk, bk, Wv, bv, Wo, bo,
                  **spmd_kwargs):
    """Build+run; returns (BassKernelResults, assembled full output)."""
    nc = _get_nc()
    in_maps = _prep_inputs(query, key_, value, Wq, bq, Wk, bk, Wv, bv, Wo, bo)
    res = run_bass_kernel_spmd(nc, in_maps, core_ids=list(range(NCORES)),
                               **spmd_kwargs)
    bo = np.asarray(bo, np.float32)
    out = np.zeros((B, S, D), np.float32)
    for c in range(NCORES):
        hg, b = divmod(c, B)
        out[b] += res.results[c]["out_part"]
    out += bo
    return res, out


def kernel(query, key_, value, Wq, bq, Wk, bk, Wv, bv, Wo, bo):
    _, out = run_on_device(query, key_, value, Wq, bq, Wk, bk, Wv, bv, Wo, bo)
    return out


# revision 11
# speedup vs baseline: 1.3418x; 1.0410x over previous
"""Multi-head attention (B=2, S=2048, D=2048, H=16) on 8 Trainium2 NeuronCores.

Sharding: 2D grid (4 head-groups x 2 batches). Core c = (hg=c//2, b=c%2)
computes heads [4*hg, 4*hg+4) of batch b end-to-end:
  - q/k/v projections for its 4 heads over its batch's 2048 tokens
    (host pre-transposes activations to [D, S] and weight slices to
    [D, 512] so every matmul contraction sits on the partition dim),
  - full attention for its 4 (b, h) pairs in transposed-score space
    (scoresT[ki,qi] -> exp -> v.T @ expT accumulation); the softmax
    denominator rides the PE as a ones-stationary matmul accumulating
    [1, qi] sums in PSUM, reshaped to per-partition scalars via an
    SBUF->SBUF DMA; blocks are software-pipelined so the PE consumes
    block B-1's exp tiles while ACT produces block B's,
  - row-parallel output projection producing a [2048, 2048] partial
    that the host sums over the 4 head-groups (plus bo).

All matmuls run as float32r (replicated-fp32; ~1.5e-4 max rel err vs
fp64, same as TRN2's fp32 path, at a fraction of the cost).
"""
import math
import numpy as np
from contextlib import ExitStack

import concourse.bacc as bacc
import concourse.mybir as mybir
import concourse.tile as tile
from concourse.bass_utils import run_bass_kernel_spmd
from concourse.masks import make_identity

F32 = mybir.dt.float32
F32R = mybir.dt.float32r
BF16 = mybir.dt.bfloat16
AF = mybir.ActivationFunctionType
AX = mybir.AxisListType

B, S, D, H = 2, 2048, 2048, 16
HD = D // H            # 128
NCORES = 8
HGROUPS = 4            # head groups
NH = H // HGROUPS      # 4 heads per core
FEAT = NH * HD         # 512 projected features per core
TOK = S                # tokens per core (one batch)
DT = D // 128          # 16 contraction tiles
IT = TOK // 128        # 16 token tiles
IB = 512               # i-block width (projections / out-proj moving dim)
NB = TOK // IB         # 4 i-blocks
QB = 512               # qi-block width (attention moving dim)
NQ = TOK // QB         # 4 qi-blocks
KT = TOK // 128        # 16 ki tiles
QC = QB // 128         # 4 qi chunks per block

_NC_CACHE = {}


def _build_nc():
    nc = bacc.Bacc("TRN2", target_bir_lowering=False, debug=False)

    xq = nc.dram_tensor("xq", [D, TOK], F32R, kind="ExternalInput").ap()
    xk = nc.dram_tensor("xk", [D, TOK], F32R, kind="ExternalInput").ap()
    xv = nc.dram_tensor("xv", [D, TOK], F32R, kind="ExternalInput").ap()
    wq = nc.dram_tensor("wq", [D, FEAT], F32R, kind="ExternalInput").ap()
    wk = nc.dram_tensor("wk", [D, FEAT], F32R, kind="ExternalInput").ap()
    wv = nc.dram_tensor("wv", [D, FEAT], F32R, kind="ExternalInput").ap()
    wo = nc.dram_tensor("wo", [FEAT, D], F32R, kind="ExternalInput").ap()
    bqs = nc.dram_tensor("bqs", [128, NH], F32, kind="ExternalInput").ap()
    bks = nc.dram_tensor("bks", [128, NH], F32, kind="ExternalInput").ap()
    bvb = nc.dram_tensor("bvb", [128, FEAT], F32, kind="ExternalInput").ap()
    out_part = nc.dram_tensor("out_part", [TOK, D], F32, kind="ExternalOutput").ap()

    with tile.TileContext(nc) as tc, ExitStack() as ctx:
        wide = ctx.enter_context(tc.tile_pool(name="wide", bufs=9))
        stream = ctx.enter_context(tc.tile_pool(name="stream", bufs=18))
        vpool = ctx.enter_context(tc.tile_pool(name="vpool", bufs=16))
        wpool = ctx.enter_context(tc.tile_pool(name="wpool", bufs=16))
        mpool = ctx.enter_context(tc.tile_pool(name="mpool", bufs=1))
        spool = ctx.enter_context(tc.tile_pool(name="spool", bufs=2))
        opool = ctx.enter_context(tc.tile_pool(name="opool", bufs=2))
        psum = ctx.enter_context(tc.tile_pool(name="psum", bufs=4, space="PSUM"))

        ident = mpool.tile([128, 128], F32, tag="ident")
        make_identity(nc, ident[:])
        ones = mpool.tile([128, 1], BF16, tag="ones")
        nc.vector.memset(ones[:], 1.0)
        bq_sb = mpool.tile([128, NH], F32, tag="bq")
        nc.sync.dma_start(bq_sb[:], bqs[:])
        bk_sb = mpool.tile([128, NH], F32, tag="bk")
        nc.sync.dma_start(bk_sb[:], bks[:])
        bv_sb = mpool.tile([128, FEAT], F32, tag="bv")
        nc.sync.dma_start(bv_sb[:], bvb[:])

        # PE warmup: junk matmuls with no data deps keep the HAM activity
        # monitor at full clock while the first DMAs land.
        wu = psum.tile([128, 128], F32, tag="sc", name="wu")
        for _ in range(24):
            nc.tensor.matmul(wu[:], ident[:], ident[:], start=True, stop=True)

        # ---- phase 1: projections -------------------------------------
        qT = []
        kT = []
        v_tiles = []

        for which, xdram, wdram in (("q", xq, wq), ("k", xk, wk), ("v", xv, wv)):
            w_sb = []
            for dt_i in range(DT):
                w_t = wpool.tile([128, FEAT], F32R, tag="w", name=f"w{which}{dt_i}")
                nc.sync.dma_start(w_t[:], wdram[dt_i * 128:(dt_i + 1) * 128, :])
                w_sb.append(w_t)
            if which in ("q", "k"):
                dst = qT if which == "q" else kT
                for j in range(NH):
                    t = wide.tile([128, TOK], F32R, tag="wide", name=f"{which}T{j}")
                    dst.append(t)

            for blk in range(NB):
                xt = []
                for dt_i in range(DT):
                    x_t = stream.tile([128, IB], F32R, tag="s5",
                                      name=f"x{which}{blk}_{dt_i}")
                    nc.sync.dma_start(
                        x_t[:], xdram[dt_i * 128:(dt_i + 1) * 128,
                                      blk * IB:(blk + 1) * IB])
                    xt.append(x_t)
                # d-tile-outer with 4 concurrent PSUM chains: the first
                # matmul needs only (w[0], xt[0]), so DMA streams ahead of
                # the PE instead of gating each block on 32 transfers.
                if which in ("q", "k"):
                    # out tile [feat128, IB] = sum_d W[d,jslice].T @ xT[d,blk]
                    bias = bq_sb if which == "q" else bk_sb
                    scale = (1.0 / math.sqrt(HD)) if which == "q" else 1.0
                    pss = [psum.tile([128, IB], F32, tag="sc",
                                     name=f"ps{which}{blk}{j}")
                           for j in range(NH)]
                    for dt_i in range(DT):
                        for j in range(NH):
                            nc.tensor.matmul(
                                pss[j][:],
                                w_sb[dt_i][:, j * 128:(j + 1) * 128],
                                xt[dt_i][:],
                                start=(dt_i == 0), stop=(dt_i == DT - 1))
                    for j in range(NH):
                        nc.vector.tensor_scalar(
                            dst[j][:, blk * IB:(blk + 1) * IB], pss[j][:],
                            float(scale), bias[:, j:j + 1],
                            op0=mybir.AluOpType.mult, op1=mybir.AluOpType.add)
                else:
                    # v: out tile [tok128, FEAT] = sum_d xT[d,itile].T @ W[d,:]
                    pss = [psum.tile([128, FEAT], F32, tag="sc",
                                     name=f"psv{blk}{it_l}")
                           for it_l in range(IB // 128)]
                    for dt_i in range(DT):
                        for it_l in range(IB // 128):
                            nc.tensor.matmul(
                                pss[it_l][:],
                                xt[dt_i][:, it_l * 128:(it_l + 1) * 128],
                                w_sb[dt_i][:],
                                start=(dt_i == 0), stop=(dt_i == DT - 1))
                    for it_l in range(IB // 128):
                        v_t = vpool.tile([128, FEAT], BF16, tag="v",
                                         name=f"v{blk}{it_l}")
                        nc.vector.tensor_add(v_t[:], pss[it_l][:], bv_sb[:])
                        v_tiles.append(v_t)

        # ---- phase 2: attention, software-pipelined over qi-blocks ----
        ctxT = [wide.tile([128, TOK], F32R, tag="wide", name=f"ctxT{h}")
                for h in range(NH)]

        def block_pass(cur, prev):
            """Interleave at tile granularity: score+exp for block `cur`
            with attn@v + denominator matmuls consuming block `prev`'s exp
            tiles, so the PE never waits on the current block's ACT work
            and prev's exp tiles free up as the loop advances."""
            ets = []
            if cur is not None:
                h, qb = cur
                qs = qT[h][:, qb * QB:(qb + 1) * QB]
                kh = kT[h]
            if prev is not None:
                ph, pqb, pets = prev
                avp = psum.tile([128, QB], F32, tag="post", name=f"avp{ph}{pqb}")
                denp = psum.tile([1, QB], F32, tag="post", name=f"dnp{ph}{pqb}")
            for t in range(KT):
                if cur is not None:
                    sps = psum.tile([128, QB], F32, tag="sc",
                                    name=f"sp{h}{qb}{t}")
                    nc.tensor.matmul(sps[:], kh[:, t * 128:(t + 1) * 128], qs,
                                     start=True, stop=True)
                    et = stream.tile([128, QB], BF16, tag="s5",
                                     name=f"e{h}{qb}{t}")
                    nc.scalar.activation(et[:], sps[:], AF.Exp)
                    ets.append(et)
                if prev is not None:
                    nc.tensor.matmul(
                        avp[:], v_tiles[t][:, ph * 128:(ph + 1) * 128],
                        pets[t][:], start=(t == 0), stop=(t == KT - 1))
                    nc.tensor.matmul(denp[:], ones[:], pets[t][:],
                                     start=(t == 0), stop=(t == KT - 1))
            if prev is None:
                return ets
            # posts for prev: denominator reshape + normalize + ctxT
            den_sb = spool.tile([1, QB], F32, tag="denr", name=f"dr{ph}{pqb}")
            nc.scalar.activation(den_sb[:], denp[:], AF.Copy)
            dct = psum.tile([128, QC], F32, tag="post", name=f"dct{ph}{pqb}")
            for c in range(QC):
                nc.tensor.transpose(dct[:, c:c + 1],
                                    den_sb[:1, c * 128:(c + 1) * 128],
                                    ident[:1, :1])
            recip = spool.tile([128, QC], F32, tag="recip", name=f"rc{ph}{pqb}")
            nc.vector.reciprocal(recip[:], dct[:])

            avsb = spool.tile([128, QB], F32, tag="avsb", name=f"av{ph}{pqb}")
            nc.vector.tensor_copy(avsb[:], avp[:])
            tp2 = psum.tile([128, QB], F32, tag="post", name=f"tp2{ph}{pqb}")
            for c in range(QC):
                nc.tensor.transpose(tp2[:, c * 128:(c + 1) * 128],
                                    avsb[:, c * 128:(c + 1) * 128], ident[:])
            csb = spool.tile([128, QB], F32, tag="ctxsb", name=f"cs{ph}{pqb}")
            for c in range(QC):
                nc.vector.tensor_scalar_mul(csb[:, c * 128:(c + 1) * 128],
                                            tp2[:, c * 128:(c + 1) * 128],
                                            recip[:, c:c + 1])
            tp3 = psum.tile([128, QB], F32, tag="post", name=f"tp3{ph}{pqb}")
            for c in range(QC):
                nc.tensor.transpose(tp3[:, c * 128:(c + 1) * 128],
                                    csb[:, c * 128:(c + 1) * 128], ident[:])
            nc.scalar.activation(ctxT[ph][:, pqb * QB:(pqb + 1) * QB], tp3[:],
                                 AF.Copy)
            return ets

        prev = None
        for h in range(NH):
            for qb in range(NQ):
                ets = block_pass((h, qb), prev)
                prev = (h, qb, ets)
        block_pass(None, prev)

        # ---- phase 3: output projection (row-parallel partial) ---------
        wo_sb = []
        for h in range(NH):
            row = []
            for jb in range(D // IB):
                w_t = wpool.tile([128, IB], F32R, tag="w", name=f"wo{h}{jb}")
                nc.sync.dma_start(
                    w_t[:], wo[h * 128:(h + 1) * 128, jb * IB:(jb + 1) * IB])
                row.append(w_t)
            wo_sb.append(row)

        NJ = D // IB
        for it_i in range(IT):
            pss = [psum.tile([128, IB], F32, tag="sc", name=f"po{it_i}{jb}")
                   for jb in range(NJ)]
            for h in range(NH):
                for jb in range(NJ):
                    nc.tensor.matmul(pss[jb][:],
                                     ctxT[h][:, it_i * 128:(it_i + 1) * 128],
                                     wo_sb[h][jb][:],
                                     start=(h == 0), stop=(h == NH - 1))
            for jb in range(NJ):
                osb = opool.tile([128, IB], F32, tag=f"o{jb % 2}",
                                 name=f"o{it_i}{jb}")
                if jb % 2 == 0:
                    nc.vector.tensor_copy(osb[:], pss[jb][:])
                else:
                    nc.scalar.activation(osb[:], pss[jb][:], AF.Copy)
                nc.sync.dma_start(
                    out_part[it_i * 128:(it_i + 1) * 128,
                             jb * IB:(jb + 1) * IB], osb[:])

    nc.compile()
    return nc


def _get_nc():
    if "nc" not in _NC_CACHE:
        _NC_CACHE["nc"] = _build_nc()
    return _NC_CACHE["nc"]


def _prep_inputs(query, key_, value, Wq, bq, Wk, bk, Wv, bv, Wo, bo):
    f32 = np.float32
    query = np.asarray(query, f32)
    key_ = np.asarray(key_, f32)
    value = np.asarray(value, f32)
    Wq, bq = np.asarray(Wq, f32), np.asarray(bq, f32)
    Wk, bk = np.asarray(Wk, f32), np.asarray(bk, f32)
    Wv, bv = np.asarray(Wv, f32), np.asarray(bv, f32)
    Wo = np.asarray(Wo, f32)

    sc = f32(1.0 / math.sqrt(HD))
    xqT = [np.ascontiguousarray(query[b].T) for b in range(B)]
    xkT = [np.ascontiguousarray(key_[b].T) for b in range(B)]
    xvT = [np.ascontiguousarray(value[b].T) for b in range(B)]

    per_hg = []
    for hg in range(HGROUPS):
        rows = slice(hg * FEAT, (hg + 1) * FEAT)
        per_hg.append(dict(
            wq=np.ascontiguousarray(Wq[rows, :].T),
            wk=np.ascontiguousarray(Wk[rows, :].T),
            wv=np.ascontiguousarray(Wv[rows, :].T),
            wo=np.ascontiguousarray(Wo[:, rows].T),
            bqs=np.ascontiguousarray((bq[rows] * sc).reshape(NH, 128).T),
            bks=np.ascontiguousarray(bk[rows].reshape(NH, 128).T),
            bvb=np.ascontiguousarray(np.broadcast_to(bv[rows], (128, FEAT))),
        ))

    in_maps = []
    for c in range(NCORES):
        hg, b = divmod(c, B)
        m = dict(per_hg[hg])
        m["xq"] = xqT[b]
        m["xk"] = xkT[b]
        m["xv"] = xvT[b]
        in_maps.append(m)
    return in_maps


def run_on_device(query, key_, value, Wq, bq, Wk, bk, Wv, bv, Wo, bo,
                  **spmd_kwargs):
    """Build+run; returns (BassKernelResults, assembled full output)."""
    nc = _get_nc()
    in_maps = _prep_inputs(query, key_, value, Wq, bq, Wk, bk, Wv, bv, Wo, bo)
    res = run_bass_kernel_spmd(nc, in_maps, core_ids=list(range(NCORES)),
                               **spmd_kwargs)
    bo = np.asarray(bo, np.float32)
    out = np.zeros((B, S, D), np.float32)
    for c in range(NCORES):
        hg, b = divmod(c, B)
        out[b] += res.results[c]["out_part"]
    out += bo
    return res, out


def kernel(query, key_, value, Wq, bq, Wk, bk, Wv, bv, Wo, bo):
    _, out = run_on_device(query, key_, value, Wq, bq, Wk, bk, Wv, bv, Wo, bo)
    return out
